# revision 5
# baseline (speedup 1.0000x reference)
"""Erosion (5x5 sliding-window min, geodesic border pad 1e4) on TRN2.

Input:  mask (32, 1, 1024, 1024) f32, values in [0, 1).
Output: same shape; out[b,0,i,j] = min over the 5x5 window centered at
        (i,j), out-of-bounds treated as 1e4.

Strategy: batch-parallel over 8 cores (4 images per core). Per image,
process 9 chunks of 124 output rows. Each chunk loads 128 input rows
(+-2 halo) into a [128, 1028] column-padded SBUF tile. Horizontal pass:
3 log-doubling tensor_tensor(min) ops along the free dim. Vertical pass
(window crosses partitions, which DVE lanes cannot do): 3 SBUF->SBUF
HWDGE DMA partition-shifted copies + 3 partition-aligned
tensor_tensor(min) ops.
"""

import numpy as np

import concourse.bacc as bacc
import concourse.mybir as mybir
import concourse.tile as tile
from concourse.bass_utils import run_bass_kernel_spmd

B, H, W = 32, 1024, 1024
N_CORES = 8
PER_CORE = B // N_CORES  # 4 images per core
PX = 2                   # erosion radius (kernel is 5x5 ones)
PAD_VAL = 1e4
CHUNK = 124              # output rows per chunk
F32 = mybir.dt.float32
MIN = mybir.AluOpType.min

_CACHE = {}


def build_nc():
    nc = bacc.Bacc("TRN2", debug=False, num_devices=N_CORES)
    x = nc.dram_tensor("mask", [PER_CORE, H, W], F32, kind="ExternalInput").ap()
    y = nc.dram_tensor("out", [PER_CORE, H, W], F32, kind="ExternalOutput").ap()

    n_chunks = (H + CHUNK - 1) // CHUNK  # 9

    with tile.TileContext(nc) as tc:
        with (
            tc.tile_pool(name="io", bufs=3) as io_pool,
            tc.tile_pool(name="tmp", bufs=2) as tmp_pool,
        ):
            for b in range(PER_CORE):
                for c in range(n_chunks):
                    r0 = c * CHUNK          # first output row
                    lo = r0 - PX            # first input row in tile (may be <0)
                    rows_out = min(CHUNK, H - r0)

                    t = io_pool.tile([128, W + 2 * PX], F32, tag="in")
                    # pad columns (left/right borders of every row)
                    nc.vector.memset(t[:, 0:PX], PAD_VAL)
                    nc.vector.memset(t[:, W + PX : W + 2 * PX], PAD_VAL)
                    # pad rows (image top/bottom)
                    p_start = max(lo, 0) - lo          # first real partition
                    n_real = min(lo + 128, H) - max(lo, 0)
                    if p_start > 0:
                        nc.vector.memset(t[0:p_start, :], PAD_VAL)
                    if p_start + n_real < 128:
                        # compute ops may only start at partition 0/32/64/96
                        # (max 128/32/64/32 partitions): round down to the
                        # previous boundary and emit legal segments; the DMA
                        # below overwrites any real rows in the overlap.
                        ms = (p_start + n_real) // 32 * 32
                        segs = {0: [(0, 128)], 32: [(32, 64), (64, 128)],
                                64: [(64, 128)], 96: [(96, 128)]}[ms]
                        for s, e in segs:
                            nc.vector.memset(t[s:e, :], PAD_VAL)
                    nc.sync.dma_start(
                        out=t[p_start : p_start + n_real, PX : W + PX],
                        in_=x[b, max(lo, 0) : max(lo, 0) + n_real, :],
                    )

                    # horizontal pass: h[j] = min(t[j .. j+4]), j = 0..W-1
                    a = tmp_pool.tile([128, W + 3], F32, tag="a")
                    nc.vector.tensor_tensor(
                        out=a[:, :], in0=t[:, 0 : W + 3], in1=t[:, 1 : W + 4], op=MIN
                    )
                    bb = tmp_pool.tile([128, W + 1], F32, tag="b")
                    nc.vector.tensor_tensor(
                        out=bb[:, :], in0=a[:, 0 : W + 1], in1=a[:, 2 : W + 3], op=MIN
                    )
                    h = tmp_pool.tile([128, W], F32, tag="h")
                    nc.vector.tensor_tensor(
                        out=h[:, :], in0=bb[:, 0:W], in1=t[:, 4 : W + 4], op=MIN
                    )

                    # vertical pass: out[p] = min(h[p .. p+4]), p = 0..rows_out-1
                    s1 = tmp_pool.tile([128, W], F32, tag="s1")
                    nc.sync.dma_start(out=s1[0:127, :], in_=h[1:128, :])
                    w2 = tmp_pool.tile([128, W], F32, tag="w2")
                    nc.vector.tensor_tensor(
                        out=w2[0:127, :], in0=h[0:127, :], in1=s1[0:127, :], op=MIN
                    )
                    s2 = tmp_pool.tile([128, W], F32, tag="s2")
                    nc.sync.dma_start(out=s2[0:125, :], in_=w2[2:127, :])
                    w4 = tmp_pool.tile([128, W], F32, tag="w4")
                    nc.vector.tensor_tensor(
                        out=w4[0:125, :], in0=w2[0:125, :], in1=s2[0:125, :], op=MIN
                    )
                    s4 = tmp_pool.tile([128, W], F32, tag="s4")
                    nc.sync.dma_start(out=s4[0:124, :], in_=h[4:128, :])
                    o = tmp_pool.tile([128, W], F32, tag="o")
                    nc.vector.tensor_tensor(
                        out=o[0:rows_out, :],
                        in0=w4[0:rows_out, :],
                        in1=s4[0:rows_out, :],
                        op=MIN,
                    )
                    nc.sync.dma_start(
                        out=y[b, r0 : r0 + rows_out, :], in_=o[0:rows_out, :]
                    )

    nc.compile()
    return nc


def run(mask: np.ndarray, trace: bool = False):
    """Returns (output, BassKernelResults)."""
    assert mask.shape == (B, 1, H, W), mask.shape
    in_dtype = mask.dtype
    mask4 = np.ascontiguousarray(
        mask.reshape(B, H, W).astype(np.float32, copy=False)
    )

    if "nc" not in _CACHE:
        _CACHE["nc"] = build_nc()
    nc = _CACHE["nc"]

    in_maps = [
        {"mask": mask4[i * PER_CORE : (i + 1) * PER_CORE]} for i in range(N_CORES)
    ]
    res = run_bass_kernel_spmd(nc, in_maps, list(range(N_CORES)), trace=trace)
    out = np.concatenate([res.results[i]["out"] for i in range(N_CORES)], axis=0)
    return out.reshape(B, 1, H, W).astype(in_dtype, copy=False), res


def kernel(mask: np.ndarray) -> np.ndarray:
    return run(mask)[0]
